# revision 10
# baseline (speedup 1.0000x reference)
"""Trainium2 Bass kernel for nn_Encoder_66735201845341.

Computes h = sum_rows(x @ W.T) for x [500000, 256] f32, W [128, 256] f32,
returning [1, 128] f32.

Strategy (8 NeuronCores, data-parallel over rows of x):
  - Host: shard x row-wise into 8 equal shards (62500 rows), zero-pad each
    to 62592 rows (489*128) so the shard reshapes to [128, 125184] with each
    SBUF partition holding whole 256-element rows.
  - Device (per core): stream the shard through SBUF in [128, 4096] tiles
    (2 MiB DMAs), elementwise-accumulate on the Vector engine into a
    [128, 4096] accumulator (every free-dim index j corresponds to column
    j mod 256), fold 4096 -> 256 with a small add tree, collapse the
    partition axis with ones-matmuls on the Tensor engine, then project
    through W.T (host-pretransposed) with two [128]-contraction matmuls.
  - AllReduce the [1, 128] partial over the 8 cores, every core writes the
    full output.
"""

import numpy as np

N_CORES = 8
ROWS = 500000
COLS = 256
OUT = 128
P = 128
ROWS_PER_CORE = ROWS // N_CORES  # 62500
PAD_ROWS = 62592  # 489 * 128
FREE = PAD_ROWS * COLS // P  # 125184 floats per partition
F_TILE = 4096  # 2 MiB per DMA tile
ACC_W = 1024  # accumulator width; each tile is added in ACC_W-wide slices

_CACHE = {}


def _build(use_collective=True, repeat=1, num_devices=N_CORES, tail_repeat=1):
    import concourse.bacc as bacc
    import concourse.mybir as mybir
    from concourse.tile import TileContext

    dt = mybir.dt.float32
    nc = bacc.Bacc(
        "TRN2", target_bir_lowering=False, debug=False, num_devices=num_devices
    )
    xs = nc.dram_tensor("xs", [P, FREE], dt, kind="ExternalInput")
    wt = nc.dram_tensor("wt", [COLS, OUT], dt, kind="ExternalInput")
    y = nc.dram_tensor("y", [1, OUT], dt, kind="ExternalOutput")

    offs = []
    o = 0
    while o < FREE:
        f = min(F_TILE, FREE - o)
        offs.append((o, f))
        o += f

    with TileContext(nc) as tc:
        with (
            tc.tile_pool(name="xt", bufs=4) as xpool,
            tc.tile_pool(name="work", bufs=1) as wpool,
            tc.tile_pool(name="psum", bufs=1, space="PSUM") as ppool,
            tc.tile_pool(name="dram", bufs=1, space="DRAM") as dpool,
        ):
            wt0 = wpool.tile([P, OUT], dt, tag="wt0")
            wt1 = wpool.tile([P, OUT], dt, tag="wt1")
            nc.sync.dma_start(wt0[:], wt[0:P, :])
            nc.sync.dma_start(wt1[:], wt[P:COLS, :])
            ones = wpool.tile([P, 1], dt, tag="ones")
            nc.vector.memset(ones[:], 1.0)

            acc = wpool.tile([P, ACC_W], dt, tag="acc")
            first = True
            for rep in range(repeat):
                for i, (o, f) in enumerate(offs):
                    xt = xpool.tile([P, F_TILE], dt, tag="xt")
                    nc.sync.dma_start(xt[:, :f], xs[:, o : o + f])
                    for j in range(0, f, ACC_W):
                        s = min(ACC_W, f - j)
                        if first:
                            nc.vector.tensor_copy(acc[:, :s], xt[:, :s])
                            first = False
                        else:
                            nc.vector.tensor_add(
                                acc[:, :s], acc[:, :s], xt[:, j : j + s]
                            )

            for _tail_rep in range(tail_repeat):
                # Fold the accumulator down to one 256-wide column sum.
                w = ACC_W
                cur = acc
                while w > 256:
                    nxt = wpool.tile([P, w // 2], dt, tag=f"t{w}")
                    nc.vector.tensor_add(
                        nxt[:], cur[:, : w // 2], cur[:, w // 2 : w]
                    )
                    cur = nxt
                    w //= 2

                # Collapse partitions: colsumT[i, h] = sum_p cur[p, h*128+i].
                pm = ppool.tile([P, 2], dt, tag="cs")
                for h in range(2):
                    nc.tensor.matmul(
                        pm[:, h : h + 1],
                        cur[:, h * 128 : (h + 1) * 128],
                        ones[:],
                        start=True,
                        stop=True,
                    )
                cb = wpool.tile([P, 2], dt, tag="csb")
                nc.vector.tensor_copy(cb[:], pm[:])

                # h[o] = sum_i colsum[i] * Wt[i, o], two K=128 contractions.
                hp = ppool.tile([1, OUT], dt, tag="h")
                nc.tensor.matmul(hp[:], cb[:, 0:1], wt0[:], start=True, stop=False)
                nc.tensor.matmul(hp[:], cb[:, 1:2], wt1[:], start=False, stop=True)
                hs = wpool.tile([1, OUT], dt, tag="hs")
                nc.vector.tensor_copy(hs[:], hp[:])

                if use_collective:
                    ib = dpool.tile([1, OUT], dt, tag="ib")
                    ob = dpool.tile([1, OUT], dt, tag="ob")
                    nc.sync.dma_start(ib[:], hs[:])
                    nc.gpsimd.collective_compute(
                        "AllReduce",
                        mybir.AluOpType.add,
                        replica_groups=[list(range(N_CORES))],
                        ins=[ib.opt()],
                        outs=[ob.opt()],
                    )
                    nc.sync.dma_start(y[:], ob[:])
                else:
                    nc.sync.dma_start(y[:], hs[:])
    nc.compile()
    return nc


def _get_nc(use_collective=True):
    key = ("nc", use_collective)
    if key not in _CACHE:
        _CACHE[key] = _build(use_collective)
    return _CACHE[key]


def _build_repeat(repeat):
    """Timing-only variant: run the bulk pass `repeat` times in one NEFF."""
    return _build(use_collective=True, repeat=repeat)


def _build_tail_repeat(tail_repeat):
    """Timing-only variant: one bulk pass, tail repeated `tail_repeat` times."""
    return _build(use_collective=True, tail_repeat=tail_repeat)


def make_in_maps(x, W):
    x = np.asarray(x, dtype=np.float32)
    W = np.asarray(W, dtype=np.float32)
    wt = np.ascontiguousarray(W.T)  # [256, 128]
    in_maps = []
    for c in range(N_CORES):
        shard = np.zeros((PAD_ROWS, COLS), dtype=np.float32)
        shard[:ROWS_PER_CORE] = x[c * ROWS_PER_CORE : (c + 1) * ROWS_PER_CORE]
        in_maps.append({"xs": shard.reshape(P, FREE), "wt": wt})
    return in_maps


def kernel(x, W):
    from concourse.bass_utils import run_bass_kernel_spmd

    nc = _get_nc(True)
    in_maps = make_in_maps(x, W)
    for attempt in range(3):
        res = run_bass_kernel_spmd(nc, in_maps, core_ids=list(range(N_CORES)))
        ys = [r["y"] for r in res.results]
        # Every core holds the identical all-reduced result. Disagreement, or
        # an all-zero result for nonzero input, indicates a transient
        # execution failure (PJRT returns the donated zero buffer) — retry.
        agree = all(np.array_equal(ys[0], yc) for yc in ys[1:])
        degenerate = not np.any(ys[0])
        if agree and not degenerate:
            return ys[0]
    return ys[0]
